# revision 8
# baseline (speedup 1.0000x reference)
"""Trainium2 Bass kernel for an 8-expert MoE FFN layer (nn_MoELayer).

Reference computation (per expert e over its contiguous 1024-token chunk):
    h = gelu(x_e @ w1[e] + b1[e]);  y_e = h @ w2[e] + b2[e]

Sharding: expert parallelism — core e holds expert e's weights and its token
chunk (the gate yields equal contiguous chunks, so no all-to-all is needed).
Each core runs the same SPMD program on its own data.

Per-core kernel (T=1024 tokens, D=1024, F=4096), all matmuls in fp16 with
fp32 PSUM accumulation (~216 ns per 512-wide matmul, the PE floor):

  phase 1: for each 128-wide f-tile: h^T[ft] = gelu(w1[:,ft]^T @ x^T + b1[ft])
           (f on partitions -> b1 is a per-partition ACT bias; h^T resident in
           SBUF).  While phase 1 runs, the vector engine (otherwise idle)
           builds the Strassen B-side combos of h for phase 2.

  phase 2: y^T = w2^T h^T + b2 via one level of Strassen-Winograd over the
           block split [D/2, F/2, T/2]: 7 products Mi of shape [512, 512]
           instead of 8 — 448 matmuls instead of 512.  The A-side (w2) combos
           are prepared on the host; the B-side combos T1..T4 of h are built
           by the vector engine during phase 1; the C-side assembly runs on
           the vector engine (tensor_tensor + fused scalar_tensor_tensor with
           the b2 bias), overlapped with the PE.

Head: the first ~1.3 MiB (w1[0] + x chunk 0) gates the matmul stream, and the
DMA path is latency-bound (~200-250 GB/s) for the first few microseconds.
The critical pieces stream on the sync/SP HWDGE ring in fine slices (per-DMA
completion semaphores let each matmul start as its slice lands); secondary
pieces ride the gpsimd SWDGE ring and the scalar/ACT HWDGE ring (which has a
~2.8 us cold-start lag).  Dummy matmuls on scratch bridge the PE from the
preamble barrier to the first real matmul and open the HAM clock-warmup
window early.

Tail: the last product of the last Strassen unit runs as two 256-wide PSUM
groups so only a 256-column assembly+flush remains after the final matmul.
"""

import os

import numpy as np

# The kernel executes through the axon PJRT backend; a CPU pin (e.g. set for
# a jax reference run) would break NEFF dispatch in this process.
if os.environ.get("JAX_PLATFORMS") == "cpu":
    del os.environ["JAX_PLATFORMS"]

E = 8          # experts == cores
B, S = 2, 4096
D = 1024       # d_model
F = 4096       # d_ff
T = (B * S) // E  # tokens per expert chunk = 1024
P = 128
DO = D // P    # 8  k-tiles of d_model
FT = F // P    # 32 f-tiles of d_ff
DMO = D // P   # 8  output dm-tiles
FH = FT // 2   # 16 f-tiles per Strassen half of d_ff
NCHUNK = T // 512  # 2 moving-operand chunks (PSUM bank caps matmul N at 512)
N_WARMUP_MM = 6

_cached = None


def _build():
    import concourse.mybir as mybir
    import concourse.tile as tile
    from concourse import bacc
    from concourse.tile_rust import add_dep_helper

    f32 = mybir.dt.float32
    f16 = mybir.dt.float16
    add = mybir.AluOpType.add
    sub = mybir.AluOpType.subtract

    nc = bacc.Bacc("TRN2", target_bir_lowering=False, debug=False, num_devices=E)

    xT_d = nc.dram_tensor("xT", [NCHUNK, P, DO, 512], f16, kind="ExternalInput")
    w1_d = nc.dram_tensor("w1r", [FT, P, DO, P], f16, kind="ExternalInput")
    bc_d = nc.dram_tensor("bc", [P, FT + DMO], f32, kind="ExternalInput")
    # Strassen A-side operands for phase 2: [mi, j, p(f-within-half), k, dm]
    w2_d = nc.dram_tensor("w2s", [7, 4, P, FH, P], f16, kind="ExternalInput")
    yT_d = nc.dram_tensor("yT", [DMO, P, T], f32, kind="ExternalOutput")

    gelu = mybir.ActivationFunctionType.Gelu_apprx_tanh

    with tile.TileContext(nc) as tc:
        with (
            tc.tile_pool(name="xpool", bufs=1) as xpool,
            tc.tile_pool(name="hpool", bufs=1) as hpool,
            tc.tile_pool(name="tpool", bufs=1) as tpool,
            tc.tile_pool(name="wpool", bufs=2) as wpool,
            tc.tile_pool(name="cpool", bufs=1) as cpool,
            tc.tile_pool(name="spool", bufs=2) as spool,
            tc.tile_pool(name="ypool", bufs=2) as ypool,
            tc.tile_pool(name="psum_h", bufs=2, space="PSUM") as psum_h,
            tc.tile_pool(name="psum_y", bufs=2, space="PSUM") as psum_y,
        ):
            # scratch for PE warmup: direct f16 memset on the vector engine
            scratch = cpool.tile([P, 512], f16)
            nc.vector.memset(scratch[:], 0.0)

            # ---- head input DMAs.  Early DMA throughput is latency-bound at
            # ~200-250 GB/s TOTAL across all rings (the SDMA engines are
            # shared, packets round-robin across queues), so priority comes
            # from keeping the early window exclusive: the critical sequence
            # w1[0] -> x chunk-0 slices streams alone on the sync/SP ring;
            # every secondary piece (gpsimd SWDGE / scalar ACT rings) is
            # gated behind an x-slice completion so it cannot steal early
            # bandwidth.  Per-slice completion sems let each matmul start as
            # its slice lands.
            w1_tiles = {}
            for ft in range(5):
                w1_tiles[ft] = wpool.tile(
                    [P, DO, P], f16, tag="w1", bufs=8, name="w1_sb"
                )
            xT_sb = xpool.tile([P, NCHUNK, DO * 512], f16)
            xc0 = xT_d.ap()[0].rearrange("p do t -> p (do t)")
            xc1 = xT_d.ap()[1].rearrange("p do t -> p (do t)")
            QX = DO * 512 // 4  # do-pair slice: 1024 fp16 per partition

            bc_sb = cpool.tile([P, FT + DMO], f32)
            nc.sync.dma_start(w1_tiles[0][:], w1_d.ap()[0])
            xs = []
            for q in range(4):
                xs.append(
                    nc.sync.dma_start(
                        xT_sb[:, 0, q * QX : (q + 1) * QX],
                        xc0[:, q * QX : (q + 1) * QX],
                    )
                )
            # biases are tiny — let them ride the idle SWDGE ring immediately
            nc.gpsimd.dma_start(bc_sb[:], bc_d.ap())
            for dma, dep in (
                (nc.gpsimd.dma_start(w1_tiles[1][:], w1_d.ap()[1]), xs[1]),
                (nc.gpsimd.dma_start(w1_tiles[2][:], w1_d.ap()[2]), xs[2]),
                (nc.scalar.dma_start(w1_tiles[3][:], w1_d.ap()[3]), xs[1]),
                (nc.scalar.dma_start(w1_tiles[4][:], w1_d.ap()[4]), xs[2]),
            ):
                add_dep_helper(
                    dma.ins, dep.ins, sync=True,
                    reason="keep the early DMA window exclusive to w1[0]+xc0",
                )
            for q in range(4):
                nc.sync.dma_start(
                    xT_sb[:, 1, q * QX : (q + 1) * QX], xc1[:, q * QX : (q + 1) * QX]
                )
            b1_sb = bc_sb[:, :FT]
            b2_sb = bc_sb[:, FT:]

            # PE warmup: dummy matmuls on scratch while the head DMAs stream.
            for i in range(N_WARMUP_MM):
                pw = psum_y.tile([P, 512], f32, tag="pm", bufs=4, name="pwarm")
                nc.tensor.matmul(
                    pw[:], scratch[:, :P], scratch[:], start=True, stop=True
                )

            h_sb = hpool.tile([P, FT, T], f16)
            # Strassen B-side combos of h: [P, combo(T1,T2,T3,T4), k, 512]
            t_sb = tpool.tile([P, 4, FH, 512], f16)

            # ---- phase 1: h^T = gelu(w1^T x^T + b1), one 128-row f-tile at a time
            def mm1_group(ph, w1_sb, c):
                for do in range(DO):
                    nc.tensor.matmul(
                        ph[:],
                        w1_sb[:, do, :],
                        xT_sb[:, c, do * 512 : (do + 1) * 512],
                        start=(do == 0),
                        stop=(do == DO - 1),
                    )

            def gelu_chunk(ph, ft, c):
                cs = slice(c * 512, (c + 1) * 512)
                return nc.scalar.activation(
                    h_sb[:, ft, cs], ph[:], gelu, bias=b1_sb[:, ft : ft + 1]
                )

            def t_combos(ft):
                # B11[i]=h[i,c0] B12[i]=h[i,c1] B21[i]=h[16+i,c0] B22[i]=h[16+i,c1]
                # T1=B12-B11 (at ft=i<16); T3=B22-B12, T2=B22-T1, T4=T2-B21
                # (at ft=16+i) — all on the otherwise-idle vector engine.
                if ft < FH:
                    i = ft
                    nc.vector.tensor_sub(
                        t_sb[:, 0, i, :], h_sb[:, i, 512:1024], h_sb[:, i, 0:512]
                    )
                else:
                    i = ft - FH
                    nc.vector.tensor_sub(
                        t_sb[:, 2, i, :], h_sb[:, ft, 512:1024], h_sb[:, i, 512:1024]
                    )
                    nc.vector.tensor_sub(
                        t_sb[:, 1, i, :], h_sb[:, ft, 512:1024], t_sb[:, 0, i, :]
                    )
                    nc.vector.tensor_sub(
                        t_sb[:, 3, i, :], t_sb[:, 1, i, :], h_sb[:, ft, 0:512]
                    )

            gelu_insts = {}
            HEAD = 3
            head_ph = {}
            for ft in range(HEAD):
                ph = psum_h.tile([P, 512], f32, tag="ph", bufs=4, name="ph")
                head_ph[ft] = ph
                mm1_group(ph, w1_tiles[ft], 0)
            for ft in range(HEAD):
                ph = head_ph[ft]
                gelu_insts[(ft, 0)] = gelu_chunk(ph, ft, 0)
                ph2 = psum_h.tile([P, 512], f32, tag="ph", bufs=4, name="ph")
                mm1_group(ph2, w1_tiles[ft], 1)
                gelu_insts[(ft, 1)] = gelu_chunk(ph2, ft, 1)
                t_combos(ft)

            for ft in range(HEAD, FT):
                if ft >= 5:
                    w1_tiles[ft] = wpool.tile(
                        [P, DO, P], f16, tag="w1", bufs=8, name="w1_sb"
                    )
                    nc.sync.dma_start(w1_tiles[ft][:], w1_d.ap()[ft])
                w1_sb = w1_tiles[ft]
                for c in range(NCHUNK):
                    ph = psum_h.tile([P, 512], f32, tag="ph", bufs=4, name="ph")
                    mm1_group(ph, w1_sb, c)
                    gelu_insts[(ft, c)] = gelu_chunk(ph, ft, c)
                t_combos(ft)

            # ---- phase 2: y^T = w2^T h^T + b2 via Strassen-Winograd.
            # Per unit j (output dm-tiles j and 4+j):
            #   M1=A11 B11  M2=A12 B21  M3=S4 B22  M4=A22 T4
            #   M5=S1 T1    M6=S2 T2    M7=S3 T3
            #   U2=M1+M6  U3=U2+M7  U4=U2+M5
            #   C11=M1+M2  C12=U4+M3  C21=U3-M4  C22=U3+M5   (+b2 folded in)
            # Mi order [1,2,6,7,5,3,4] releases each PSUM bank right after
            # its product completes.
            def moving(mi, k):
                if mi == 0:
                    return h_sb[:, k, 0:512]
                if mi == 1:
                    return h_sb[:, FH + k, 0:512]
                if mi == 2:
                    return h_sb[:, FH + k, 512:1024]
                combo = {4: 0, 5: 1, 6: 2, 3: 3}[mi]  # M5->T1 M6->T2 M7->T3 M4->T4
                return t_sb[:, combo, k, :]

            def mi_product(mi, j, n0=0, n1=512, pm=None):
                # accumulate Mi[j][:, n0:n1] over the 16 k-tiles; weights
                # stream in two [P, 8, P] slabs through the shared w1 pool
                if pm is None:
                    pm_t = psum_y.tile([P, 512], f32, tag="pm", bufs=4, name="pm")
                    pm = pm_t[:, n0:n1]
                for half in range(2):
                    slab = wpool.tile([P, FH // 2, P], f16, tag="w1", bufs=8,
                                      name="w2s_sb")
                    dma = nc.sync.dma_start(
                        slab[:],
                        w2_d.ap()[mi, j][:, half * 8 : (half + 1) * 8, :],
                    )
                    if mi == 0 and j == 0:
                        add_dep_helper(
                            dma.ins,
                            gelu_insts[(6, 1)].ins,
                            sync=True,
                            reason="delay w2 prefetch past the kernel head",
                        )
                    for kk in range(FH // 2):
                        k = half * 8 + kk
                        nc.tensor.matmul(
                            pm,
                            slab[:, kk, :],
                            moving(mi, k)[:, n0:n1],
                            start=(k == 0),
                            stop=(k == FH - 1),
                        )
                return pm

            def flush(src_sb, j_out, cs):
                nc.sync.dma_start(yT_d.ap()[j_out][:, cs], src_sb[:])

            for j in range(4):
                b2j = b2_sb[:, j : j + 1]
                b2j4 = b2_sb[:, 4 + j : 5 + j]
                pm1 = mi_product(0, j)
                s_m1 = spool.tile([P, 512], f32, tag="sm1", bufs=2, name="s_m1")
                nc.vector.tensor_copy(s_m1[:], pm1)
                pm2 = mi_product(1, j)
                y11 = ypool.tile([P, 512], f32, tag="y", bufs=4, name="y_sb")
                nc.vector.scalar_tensor_tensor(y11[:], s_m1[:], b2j, pm2, add, add)
                flush(y11, j, slice(0, 512))
                pm6 = mi_product(5, j)
                u2 = spool.tile([P, 512], f32, tag="u2", bufs=2, name="u2")
                nc.vector.tensor_add(u2[:], s_m1[:], pm6)
                pm7 = mi_product(6, j)
                u3 = spool.tile([P, 512], f32, tag="u3", bufs=2, name="u3")
                nc.vector.tensor_add(u3[:], u2[:], pm7)
                pm5 = mi_product(4, j)
                u4 = spool.tile([P, 512], f32, tag="u4", bufs=2, name="u4")
                nc.vector.tensor_add(u4[:], u2[:], pm5)
                y22 = ypool.tile([P, 512], f32, tag="y", bufs=4, name="y_sb")
                nc.vector.scalar_tensor_tensor(y22[:], u3[:], b2j4, pm5, add, add)
                flush(y22, 4 + j, slice(512, 1024))
                pm3 = mi_product(2, j)
                y12 = ypool.tile([P, 512], f32, tag="y", bufs=4, name="y_sb")
                nc.vector.scalar_tensor_tensor(y12[:], u4[:], b2j, pm3, add, add)
                flush(y12, j, slice(512, 1024))
                if j < 3:
                    pm4 = mi_product(3, j)
                    y21 = ypool.tile([P, 512], f32, tag="y", bufs=4, name="y_sb")
                    nc.vector.scalar_tensor_tensor(
                        y21[:], u3[:], b2j4, pm4, add, sub
                    )
                    flush(y21, 4 + j, slice(0, 512))
                else:
                    # last unit: M4 as two 256-wide groups so only a 256-col
                    # assembly+flush remains after the final matmul
                    pm4a_t = psum_y.tile([P, 512], f32, tag="pm", bufs=4, name="pm4a")
                    pm4a = pm4a_t[:, 0:256]
                    mi_product(3, j, 0, 256, pm4a)
                    y21a = ypool.tile([P, 256], f32, tag="yh", bufs=2, name="y21a")
                    nc.vector.scalar_tensor_tensor(
                        y21a[:], u3[:, 0:256], b2j4, pm4a, add, sub
                    )
                    flush(y21a, 4 + j, slice(0, 256))
                    pm4b_t = psum_y.tile([P, 512], f32, tag="pm", bufs=4, name="pm4b")
                    pm4b = pm4b_t[:, 0:256]
                    mi_product(3, j, 256, 512, pm4b)
                    y21b = ypool.tile([P, 256], f32, tag="yh", bufs=2, name="y21b")
                    nc.vector.scalar_tensor_tensor(
                        y21b[:], u3[:, 256:512], b2j4, pm4b, add, sub
                    )
                    nc.scalar.dma_start(yT_d.ap()[4 + j][:, 256:512], y21b[:])

    nc.compile()
    return nc


def _get_nc():
    global _cached
    if _cached is None:
        _cached = _build()
    return _cached


def make_in_maps(x, w1, b1, w2, b2):
    x = np.asarray(x, dtype=np.float32)
    w1 = np.asarray(w1, dtype=np.float32)
    b1 = np.asarray(b1, dtype=np.float32)
    w2 = np.asarray(w2, dtype=np.float32)
    b2 = np.asarray(b2, dtype=np.float32)

    tokens = x.reshape(E, T, D)
    in_maps = []
    for e in range(E):
        xT = np.ascontiguousarray(
            tokens[e].reshape(NCHUNK, 512, DO, P).transpose(0, 3, 2, 1)
        ).astype(np.float16)  # [c, p, do, t']
        w1r = np.ascontiguousarray(
            w1[e].reshape(DO, P, FT, P).transpose(2, 1, 0, 3)
        ).astype(np.float16)  # [ft, p, do, j]
        bc = np.ascontiguousarray(
            np.concatenate([b1[e].reshape(FT, P).T, b2[e].reshape(DMO, P).T], axis=1)
        )  # [p, ft..dmo]
        # Strassen A-side combos of w2 [F, D]:
        w2e = w2[e]
        A11 = w2e[: F // 2, : D // 2]
        A12 = w2e[F // 2 :, : D // 2]
        A21 = w2e[: F // 2, D // 2 :]
        A22 = w2e[F // 2 :, D // 2 :]
        S1 = A21 + A22
        S2 = S1 - A11
        S3 = A11 - A21
        S4 = A12 - S2
        W = np.stack([A11, A12, S4, A22, S1, S2, S3])  # [7, F/2, D/2]
        w2s = np.ascontiguousarray(
            W.reshape(7, FH, P, 4, P).transpose(0, 3, 2, 1, 4)
        ).astype(np.float16)  # [mi, j, p, k, dm]
        in_maps.append({"xT": xT, "w1r": w1r, "bc": bc, "w2s": w2s})
    return in_maps


def gather_out(results):
    out = np.empty((E, T, D), dtype=np.float32)
    for e in range(E):
        yT = results[e]["yT"]  # [dmo, p, t]
        out[e] = yT.transpose(2, 0, 1).reshape(T, D)
    return out.reshape(B, S, D)


def kernel(x, w1, b1, w2, b2):
    from concourse.bass_utils import run_bass_kernel_spmd

    nc = _get_nc()
    in_maps = make_in_maps(x, w1, b1, w2, b2)
    res = run_bass_kernel_spmd(nc, in_maps, core_ids=list(range(E)))
    return gather_out(res.results)


# revision 10
# speedup vs baseline: 1.0167x; 1.0167x over previous
"""Trainium2 Bass kernel for an 8-expert MoE FFN layer (nn_MoELayer).

Reference computation (per expert e over its contiguous 1024-token chunk):
    h = gelu(x_e @ w1[e] + b1[e]);  y_e = h @ w2[e] + b2[e]

Sharding: expert parallelism — core e holds expert e's weights and its token
chunk (the gate yields equal contiguous chunks, so no all-to-all is needed).
Each core runs the same SPMD program on its own data.

Per-core kernel (T=1024 tokens, D=1024, F=4096), all matmuls in fp16 with
fp32 PSUM accumulation (~216 ns per 512-wide matmul, the PE floor):

  phase 1: for each 128-wide f-tile: h^T[ft] = gelu(w1[:,ft]^T @ x^T + b1[ft])
           (f on partitions -> b1 is a per-partition ACT bias; h^T resident in
           SBUF).  While phase 1 runs, the vector engine (otherwise idle)
           builds the Strassen B-side combos of h for phase 2.

  phase 2: y^T = w2^T h^T + b2 via one level of Strassen-Winograd over the
           block split [D/2, F/2, T/2]: 7 products Mi of shape [512, 512]
           instead of 8 — 448 matmuls instead of 512.  The A-side (w2) combos
           are prepared on the host; the B-side combos T1..T4 of h are built
           by the vector engine during phase 1; the C-side assembly runs on
           the vector engine (tensor_tensor + fused scalar_tensor_tensor with
           the b2 bias), overlapped with the PE.

Head: the first ~1.3 MiB (w1[0] + x chunk 0) gates the matmul stream, and the
DMA path is latency-bound (~200-250 GB/s) for the first few microseconds.
The critical pieces stream on the sync/SP HWDGE ring in fine slices (per-DMA
completion semaphores let each matmul start as its slice lands); secondary
pieces ride the gpsimd SWDGE ring and the scalar/ACT HWDGE ring (which has a
~2.8 us cold-start lag).  Dummy matmuls on scratch bridge the PE from the
preamble barrier to the first real matmul and open the HAM clock-warmup
window early.

Tail: the last product of the last Strassen unit runs as two 256-wide PSUM
groups so only a 256-column assembly+flush remains after the final matmul.
"""

import os

import numpy as np

# The kernel executes through the axon PJRT backend; a CPU pin (e.g. set for
# a jax reference run) would break NEFF dispatch in this process.
if os.environ.get("JAX_PLATFORMS") == "cpu":
    del os.environ["JAX_PLATFORMS"]

E = 8          # experts == cores
B, S = 2, 4096
D = 1024       # d_model
F = 4096       # d_ff
T = (B * S) // E  # tokens per expert chunk = 1024
P = 128
DO = D // P    # 8  k-tiles of d_model
FT = F // P    # 32 f-tiles of d_ff
DMO = D // P   # 8  output dm-tiles
FH = FT // 2   # 16 f-tiles per Strassen half of d_ff
NCHUNK = T // 512  # 2 moving-operand chunks (PSUM bank caps matmul N at 512)
N_WARMUP_MM = 21

_cached = None


def _build():
    import concourse.mybir as mybir
    import concourse.tile as tile
    from concourse import bacc
    from concourse.tile_rust import add_dep_helper

    f32 = mybir.dt.float32
    f16 = mybir.dt.float16
    add = mybir.AluOpType.add
    sub = mybir.AluOpType.subtract

    nc = bacc.Bacc("TRN2", target_bir_lowering=False, debug=False, num_devices=E)

    xT_d = nc.dram_tensor("xT", [NCHUNK, P, DO, 512], f16, kind="ExternalInput")
    w1_d = nc.dram_tensor("w1r", [FT, P, DO, P], f16, kind="ExternalInput")
    bc_d = nc.dram_tensor("bc", [P, FT + DMO], f32, kind="ExternalInput")
    # Strassen A-side operands for phase 2: [mi, j, p(f-within-half), k, dm]
    w2_d = nc.dram_tensor("w2s", [7, 4, P, FH, P], f16, kind="ExternalInput")
    yT_d = nc.dram_tensor("yT", [DMO, P, T], f32, kind="ExternalOutput")

    gelu = mybir.ActivationFunctionType.Gelu_apprx_tanh

    with tile.TileContext(nc) as tc:
        with (
            tc.tile_pool(name="xpool", bufs=1) as xpool,
            tc.tile_pool(name="hpool", bufs=1) as hpool,
            tc.tile_pool(name="tpool", bufs=1) as tpool,
            tc.tile_pool(name="wpool", bufs=2) as wpool,
            tc.tile_pool(name="cpool", bufs=1) as cpool,
            tc.tile_pool(name="spool", bufs=2) as spool,
            tc.tile_pool(name="ypool", bufs=2) as ypool,
            tc.tile_pool(name="psum_h", bufs=2, space="PSUM") as psum_h,
            tc.tile_pool(name="psum_y", bufs=2, space="PSUM") as psum_y,
        ):
            # scratch for PE warmup: direct f16 memset on the vector engine
            scratch = cpool.tile([P, 512], f16)
            nc.vector.memset(scratch[:], 0.0)

            # ---- head input DMAs.  Early DMA throughput is latency-bound at
            # ~200-250 GB/s TOTAL across all rings (the SDMA engines are
            # shared, packets round-robin across queues), so priority comes
            # from keeping the early window exclusive: the critical sequence
            # w1[0] -> x chunk-0 slices streams alone on the sync/SP ring;
            # every secondary piece (gpsimd SWDGE / scalar ACT rings) is
            # gated behind an x-slice completion so it cannot steal early
            # bandwidth.  Per-slice completion sems let each matmul start as
            # its slice lands.
            w1_tiles = {}
            for ft in range(5):
                w1_tiles[ft] = wpool.tile(
                    [P, DO, P], f16, tag="w1", bufs=8, name="w1_sb"
                )
            xT_sb = xpool.tile([P, NCHUNK, DO * 512], f16)
            xc0 = xT_d.ap()[0].rearrange("p do t -> p (do t)")
            xc1 = xT_d.ap()[1].rearrange("p do t -> p (do t)")
            QX = DO * 512 // 4  # do-pair slice: 1024 fp16 per partition

            bc_sb = cpool.tile([P, FT + DMO], f32)
            nc.sync.dma_start(w1_tiles[0][:], w1_d.ap()[0])
            xs = []
            for q in range(2):
                xs.append(
                    nc.sync.dma_start(
                        xT_sb[:, 0, q * QX : (q + 1) * QX],
                        xc0[:, q * QX : (q + 1) * QX],
                    )
                )
            nc.sync.dma_start(w1_tiles[1][:], w1_d.ap()[1])
            for q in range(2, 4):
                xs.append(
                    nc.sync.dma_start(
                        xT_sb[:, 0, q * QX : (q + 1) * QX],
                        xc0[:, q * QX : (q + 1) * QX],
                    )
                )
            nc.sync.dma_start(w1_tiles[2][:], w1_d.ap()[2])
            # biases are tiny — let them ride the idle SWDGE ring immediately
            nc.gpsimd.dma_start(bc_sb[:], bc_d.ap())
            for dma, dep in (
                (nc.scalar.dma_start(w1_tiles[3][:], w1_d.ap()[3]), xs[1]),
                (nc.scalar.dma_start(w1_tiles[4][:], w1_d.ap()[4]), xs[2]),
            ):
                add_dep_helper(
                    dma.ins, dep.ins, sync=True,
                    reason="keep the early DMA window exclusive to w1[0]+xc0",
                )
            for q in range(4):
                nc.sync.dma_start(
                    xT_sb[:, 1, q * QX : (q + 1) * QX], xc1[:, q * QX : (q + 1) * QX]
                )
            b1_sb = bc_sb[:, :FT]
            b2_sb = bc_sb[:, FT:]

            # PE warmup: dummy matmuls on scratch while the head DMAs stream.
            for i in range(N_WARMUP_MM):
                pw = psum_y.tile([P, 512], f32, tag="pm", bufs=4, name="pwarm")
                nc.tensor.matmul(
                    pw[:], scratch[:, :P], scratch[:], start=True, stop=True
                )

            h_sb = hpool.tile([P, FT, T], f16)
            # Strassen B-side combos of h: [P, combo(T1,T2,T3,T4), k, 512]
            t_sb = tpool.tile([P, 4, FH, 512], f16)

            # ---- phase 1: h^T = gelu(w1^T x^T + b1), one 128-row f-tile at a time
            def mm1_group(ph, w1_sb, c):
                for do in range(DO):
                    nc.tensor.matmul(
                        ph[:],
                        w1_sb[:, do, :],
                        xT_sb[:, c, do * 512 : (do + 1) * 512],
                        start=(do == 0),
                        stop=(do == DO - 1),
                    )

            def gelu_chunk(ph, ft, c):
                cs = slice(c * 512, (c + 1) * 512)
                return nc.scalar.activation(
                    h_sb[:, ft, cs], ph[:], gelu, bias=b1_sb[:, ft : ft + 1]
                )

            def t_combos(ft):
                # B11[i]=h[i,c0] B12[i]=h[i,c1] B21[i]=h[16+i,c0] B22[i]=h[16+i,c1]
                # T1=B12-B11 (at ft=i<16); T3=B22-B12, T2=B22-T1, T4=T2-B21
                # (at ft=16+i) — all on the otherwise-idle vector engine.
                if ft < FH:
                    i = ft
                    nc.vector.tensor_sub(
                        t_sb[:, 0, i, :], h_sb[:, i, 512:1024], h_sb[:, i, 0:512]
                    )
                else:
                    i = ft - FH
                    nc.vector.tensor_sub(
                        t_sb[:, 2, i, :], h_sb[:, ft, 512:1024], h_sb[:, i, 512:1024]
                    )
                    nc.vector.tensor_sub(
                        t_sb[:, 1, i, :], h_sb[:, ft, 512:1024], t_sb[:, 0, i, :]
                    )
                    nc.vector.tensor_sub(
                        t_sb[:, 3, i, :], t_sb[:, 1, i, :], h_sb[:, ft, 0:512]
                    )

            gelu_insts = {}
            HEAD = 3
            head_ph = {}
            for ft in range(HEAD):
                ph = psum_h.tile([P, 512], f32, tag="ph", bufs=4, name="ph")
                head_ph[ft] = ph
                mm1_group(ph, w1_tiles[ft], 0)
            for ft in range(HEAD):
                ph = head_ph[ft]
                gelu_insts[(ft, 0)] = gelu_chunk(ph, ft, 0)
                ph2 = psum_h.tile([P, 512], f32, tag="ph", bufs=4, name="ph")
                mm1_group(ph2, w1_tiles[ft], 1)
                gelu_insts[(ft, 1)] = gelu_chunk(ph2, ft, 1)
                t_combos(ft)

            for ft in range(HEAD, FT):
                if ft >= 5:
                    w1_tiles[ft] = wpool.tile(
                        [P, DO, P], f16, tag="w1", bufs=8, name="w1_sb"
                    )
                    nc.sync.dma_start(w1_tiles[ft][:], w1_d.ap()[ft])
                w1_sb = w1_tiles[ft]
                for c in range(NCHUNK):
                    ph = psum_h.tile([P, 512], f32, tag="ph", bufs=4, name="ph")
                    mm1_group(ph, w1_sb, c)
                    gelu_insts[(ft, c)] = gelu_chunk(ph, ft, c)
                t_combos(ft)

            # ---- phase 2: y^T = w2^T h^T + b2 via Strassen-Winograd.
            # Per unit j (output dm-tiles j and 4+j):
            #   M1=A11 B11  M2=A12 B21  M3=S4 B22  M4=A22 T4
            #   M5=S1 T1    M6=S2 T2    M7=S3 T3
            #   U2=M1+M6  U3=U2+M7  U4=U2+M5
            #   C11=M1+M2  C12=U4+M3  C21=U3-M4  C22=U3+M5   (+b2 folded in)
            # Mi order [1,2,6,7,5,3,4] releases each PSUM bank right after
            # its product completes.
            def moving(mi, k):
                if mi == 0:
                    return h_sb[:, k, 0:512]
                if mi == 1:
                    return h_sb[:, FH + k, 0:512]
                if mi == 2:
                    return h_sb[:, FH + k, 512:1024]
                combo = {4: 0, 5: 1, 6: 2, 3: 3}[mi]  # M5->T1 M6->T2 M7->T3 M4->T4
                return t_sb[:, combo, k, :]

            def mi_product(mi, j, n0=0, n1=512, pm=None):
                # accumulate Mi[j][:, n0:n1] over the 16 k-tiles; weights
                # stream in two [P, 8, P] slabs through the shared w1 pool
                if pm is None:
                    pm_t = psum_y.tile([P, 512], f32, tag="pm", bufs=4, name="pm")
                    pm = pm_t[:, n0:n1]
                for half in range(2):
                    slab = wpool.tile([P, FH // 2, P], f16, tag="w1", bufs=8,
                                      name="w2s_sb")
                    dma = nc.sync.dma_start(
                        slab[:],
                        w2_d.ap()[mi, j][:, half * 8 : (half + 1) * 8, :],
                    )
                    if mi == 0 and j == 0:
                        add_dep_helper(
                            dma.ins,
                            gelu_insts[(6, 1)].ins,
                            sync=True,
                            reason="delay w2 prefetch past the kernel head",
                        )
                    for kk in range(FH // 2):
                        k = half * 8 + kk
                        nc.tensor.matmul(
                            pm,
                            slab[:, kk, :],
                            moving(mi, k)[:, n0:n1],
                            start=(k == 0),
                            stop=(k == FH - 1),
                        )
                return pm

            def flush(src_sb, j_out, cs):
                nc.sync.dma_start(yT_d.ap()[j_out][:, cs], src_sb[:])

            for j in range(4):
                b2j = b2_sb[:, j : j + 1]
                b2j4 = b2_sb[:, 4 + j : 5 + j]
                pm1 = mi_product(0, j)
                s_m1 = spool.tile([P, 512], f32, tag="sm1", bufs=2, name="s_m1")
                nc.vector.tensor_copy(s_m1[:], pm1)
                pm2 = mi_product(1, j)
                y11 = ypool.tile([P, 512], f32, tag="y", bufs=4, name="y_sb")
                nc.vector.scalar_tensor_tensor(y11[:], s_m1[:], b2j, pm2, add, add)
                flush(y11, j, slice(0, 512))
                pm6 = mi_product(5, j)
                u2 = spool.tile([P, 512], f32, tag="u2", bufs=2, name="u2")
                nc.vector.tensor_add(u2[:], s_m1[:], pm6)
                pm7 = mi_product(6, j)
                u3 = spool.tile([P, 512], f32, tag="u3", bufs=2, name="u3")
                nc.vector.tensor_add(u3[:], u2[:], pm7)
                pm5 = mi_product(4, j)
                u4 = spool.tile([P, 512], f32, tag="u4", bufs=2, name="u4")
                nc.vector.tensor_add(u4[:], u2[:], pm5)
                y22 = ypool.tile([P, 512], f32, tag="y", bufs=4, name="y_sb")
                nc.vector.scalar_tensor_tensor(y22[:], u3[:], b2j4, pm5, add, add)
                flush(y22, 4 + j, slice(512, 1024))
                pm3 = mi_product(2, j)
                y12 = ypool.tile([P, 512], f32, tag="y", bufs=4, name="y_sb")
                nc.vector.scalar_tensor_tensor(y12[:], u4[:], b2j, pm3, add, add)
                flush(y12, j, slice(512, 1024))
                if j < 3:
                    pm4 = mi_product(3, j)
                    y21 = ypool.tile([P, 512], f32, tag="y", bufs=4, name="y_sb")
                    nc.vector.scalar_tensor_tensor(
                        y21[:], u3[:], b2j4, pm4, add, sub
                    )
                    flush(y21, 4 + j, slice(0, 512))
                else:
                    # last unit: M4 as two 256-wide groups so only a 256-col
                    # assembly+flush remains after the final matmul
                    pm4a_t = psum_y.tile([P, 512], f32, tag="pm", bufs=4, name="pm4a")
                    pm4a = pm4a_t[:, 0:256]
                    mi_product(3, j, 0, 256, pm4a)
                    y21a = ypool.tile([P, 256], f32, tag="yh", bufs=2, name="y21a")
                    nc.vector.scalar_tensor_tensor(
                        y21a[:], u3[:, 0:256], b2j4, pm4a, add, sub
                    )
                    flush(y21a, 4 + j, slice(0, 256))
                    pm4b_t = psum_y.tile([P, 512], f32, tag="pm", bufs=4, name="pm4b")
                    pm4b = pm4b_t[:, 0:256]
                    mi_product(3, j, 256, 512, pm4b)
                    y21b = ypool.tile([P, 256], f32, tag="yh", bufs=2, name="y21b")
                    nc.vector.scalar_tensor_tensor(
                        y21b[:], u3[:, 256:512], b2j4, pm4b, add, sub
                    )
                    nc.scalar.dma_start(yT_d.ap()[4 + j][:, 256:512], y21b[:])

    nc.compile()
    return nc


def _get_nc():
    global _cached
    if _cached is None:
        _cached = _build()
    return _cached


def make_in_maps(x, w1, b1, w2, b2):
    x = np.asarray(x, dtype=np.float32)
    w1 = np.asarray(w1, dtype=np.float32)
    b1 = np.asarray(b1, dtype=np.float32)
    w2 = np.asarray(w2, dtype=np.float32)
    b2 = np.asarray(b2, dtype=np.float32)

    tokens = x.reshape(E, T, D)
    in_maps = []
    for e in range(E):
        xT = np.ascontiguousarray(
            tokens[e].reshape(NCHUNK, 512, DO, P).transpose(0, 3, 2, 1)
        ).astype(np.float16)  # [c, p, do, t']
        w1r = np.ascontiguousarray(
            w1[e].reshape(DO, P, FT, P).transpose(2, 1, 0, 3)
        ).astype(np.float16)  # [ft, p, do, j]
        bc = np.ascontiguousarray(
            np.concatenate([b1[e].reshape(FT, P).T, b2[e].reshape(DMO, P).T], axis=1)
        )  # [p, ft..dmo]
        # Strassen A-side combos of w2 [F, D]:
        w2e = w2[e]
        A11 = w2e[: F // 2, : D // 2]
        A12 = w2e[F // 2 :, : D // 2]
        A21 = w2e[: F // 2, D // 2 :]
        A22 = w2e[F // 2 :, D // 2 :]
        S1 = A21 + A22
        S2 = S1 - A11
        S3 = A11 - A21
        S4 = A12 - S2
        W = np.stack([A11, A12, S4, A22, S1, S2, S3])  # [7, F/2, D/2]
        w2s = np.ascontiguousarray(
            W.reshape(7, FH, P, 4, P).transpose(0, 3, 2, 1, 4)
        ).astype(np.float16)  # [mi, j, p, k, dm]
        in_maps.append({"xT": xT, "w1r": w1r, "bc": bc, "w2s": w2s})
    return in_maps


def gather_out(results):
    out = np.empty((E, T, D), dtype=np.float32)
    for e in range(E):
        yT = results[e]["yT"]  # [dmo, p, t]
        out[e] = yT.transpose(2, 0, 1).reshape(T, D)
    return out.reshape(B, S, D)


def kernel(x, w1, b1, w2, b2):
    from concourse.bass_utils import run_bass_kernel_spmd

    nc = _get_nc()
    in_maps = make_in_maps(x, w1, b1, w2, b2)
    res = run_bass_kernel_spmd(nc, in_maps, core_ids=list(range(E)))
    return gather_out(res.results)


# revision 12
# speedup vs baseline: 1.0213x; 1.0045x over previous
"""Trainium2 Bass kernel for an 8-expert MoE FFN layer (nn_MoELayer).

Reference computation (per expert e over its contiguous 1024-token chunk):
    h = gelu(x_e @ w1[e] + b1[e]);  y_e = h @ w2[e] + b2[e]

Sharding: expert parallelism — core e holds expert e's weights and its token
chunk (the gate yields equal contiguous chunks, so no all-to-all is needed).
Each core runs the same SPMD program on its own data.

Per-core kernel (T=1024 tokens, D=1024, F=4096), all matmuls in fp16 with
fp32 PSUM accumulation (~216 ns per 512-wide matmul, the PE floor):

  phase 1: for each 128-wide f-tile: h^T[ft] = gelu(w1[:,ft]^T @ x^T + b1[ft])
           (f on partitions -> b1 is a per-partition ACT bias; h^T resident in
           SBUF).  While phase 1 runs, the vector engine (otherwise idle)
           builds the Strassen B-side combos of h for phase 2.

  phase 2: y^T = w2^T h^T + b2 via one level of Strassen-Winograd over the
           block split [D/2, F/2, T/2]: 7 products Mi of shape [512, 512]
           instead of 8 — 448 matmuls instead of 512.  The A-side (w2) combos
           are prepared on the host; the B-side combos T1..T4 of h are built
           by the vector engine during phase 1; the C-side assembly runs on
           the vector engine (tensor_tensor + fused scalar_tensor_tensor with
           the b2 bias), overlapped with the PE.

Head: the first ~1.3 MiB (w1[0] + x chunk 0) gates the matmul stream, and the
DMA path is latency-bound (~200-250 GB/s) for the first few microseconds.
The critical pieces stream on the sync/SP HWDGE ring in fine slices (per-DMA
completion semaphores let each matmul start as its slice lands); secondary
pieces ride the gpsimd SWDGE ring and the scalar/ACT HWDGE ring (which has a
~2.8 us cold-start lag).  Dummy matmuls on scratch bridge the PE from the
preamble barrier to the first real matmul and open the HAM clock-warmup
window early.

Tail: the last product of the last Strassen unit runs as two 256-wide PSUM
groups so only a 256-column assembly+flush remains after the final matmul.
"""

import os

import numpy as np

# The kernel executes through the axon PJRT backend; a CPU pin (e.g. set for
# a jax reference run) would break NEFF dispatch in this process.
if os.environ.get("JAX_PLATFORMS") == "cpu":
    del os.environ["JAX_PLATFORMS"]

E = 8          # experts == cores
B, S = 2, 4096
D = 1024       # d_model
F = 4096       # d_ff
T = (B * S) // E  # tokens per expert chunk = 1024
P = 128
DO = D // P    # 8  k-tiles of d_model
FT = F // P    # 32 f-tiles of d_ff
DMO = D // P   # 8  output dm-tiles
FH = FT // 2   # 16 f-tiles per Strassen half of d_ff
NCHUNK = T // 512  # 2 moving-operand chunks (PSUM bank caps matmul N at 512)
N_WARMUP_MM = 16

_cached = None


def _build():
    import concourse.mybir as mybir
    import concourse.tile as tile
    from concourse import bacc
    from concourse.tile_rust import add_dep_helper

    f32 = mybir.dt.float32
    f16 = mybir.dt.float16
    add = mybir.AluOpType.add
    sub = mybir.AluOpType.subtract

    nc = bacc.Bacc("TRN2", target_bir_lowering=False, debug=False, num_devices=E)

    xT_d = nc.dram_tensor("xT", [NCHUNK, P, DO, 512], f16, kind="ExternalInput")
    w1_d = nc.dram_tensor("w1r", [FT, P, DO, P], f16, kind="ExternalInput")
    bc_d = nc.dram_tensor("bc", [P, FT + DMO], f32, kind="ExternalInput")
    # Strassen A-side operands for phase 2: [mi, j, p(f-within-half), k, dm]
    w2_d = nc.dram_tensor("w2s", [7, 4, P, FH, P], f16, kind="ExternalInput")
    yT_d = nc.dram_tensor("yT", [DMO, P, T], f32, kind="ExternalOutput")

    gelu = mybir.ActivationFunctionType.Gelu_apprx_tanh

    with tile.TileContext(nc) as tc:
        with (
            tc.tile_pool(name="xpool", bufs=1) as xpool,
            tc.tile_pool(name="hpool", bufs=1) as hpool,
            tc.tile_pool(name="tpool", bufs=1) as tpool,
            tc.tile_pool(name="wpool", bufs=2) as wpool,
            tc.tile_pool(name="cpool", bufs=1) as cpool,
            tc.tile_pool(name="spool", bufs=2) as spool,
            tc.tile_pool(name="ypool", bufs=2) as ypool,
            tc.tile_pool(name="psum_h", bufs=2, space="PSUM") as psum_h,
            tc.tile_pool(name="psum_y", bufs=2, space="PSUM") as psum_y,
        ):
            # scratch for PE warmup: direct f16 memset on the vector engine
            scratch = cpool.tile([P, 512], f16)
            nc.vector.memset(scratch[:], 0.0)

            # ---- head input DMAs.  Early DMA throughput is latency-bound at
            # ~200-250 GB/s TOTAL across all rings (the SDMA engines are
            # shared, packets round-robin across queues), so priority comes
            # from keeping the early window exclusive: the critical sequence
            # w1[0] -> x chunk-0 slices streams alone on the sync/SP ring;
            # every secondary piece (gpsimd SWDGE / scalar ACT rings) is
            # gated behind an x-slice completion so it cannot steal early
            # bandwidth.  Per-slice completion sems let each matmul start as
            # its slice lands.
            w1_tiles = {}
            for ft in range(5):
                w1_tiles[ft] = wpool.tile(
                    [P, DO, P], f16, tag="w1", bufs=8, name="w1_sb"
                )
            xT_sb = xpool.tile([P, NCHUNK, DO * 512], f16)
            xc0 = xT_d.ap()[0].rearrange("p do t -> p (do t)")
            xc1 = xT_d.ap()[1].rearrange("p do t -> p (do t)")
            QX = DO * 512 // 4  # do-pair slice: 1024 fp16 per partition

            bc_sb = cpool.tile([P, FT + DMO], f32)
            nc.sync.dma_start(w1_tiles[0][:], w1_d.ap()[0])
            xs = []
            for q in range(4):
                xs.append(
                    nc.sync.dma_start(
                        xT_sb[:, 0, q * QX : (q + 1) * QX],
                        xc0[:, q * QX : (q + 1) * QX],
                    )
                )
            nc.sync.dma_start(w1_tiles[1][:], w1_d.ap()[1])
            nc.sync.dma_start(w1_tiles[2][:], w1_d.ap()[2])
            # biases are tiny — let them ride the idle SWDGE ring immediately
            nc.gpsimd.dma_start(bc_sb[:], bc_d.ap())
            for dma, dep in (
                (nc.scalar.dma_start(w1_tiles[3][:], w1_d.ap()[3]), xs[1]),
                (nc.scalar.dma_start(w1_tiles[4][:], w1_d.ap()[4]), xs[2]),
            ):
                add_dep_helper(
                    dma.ins, dep.ins, sync=True,
                    reason="keep the early DMA window exclusive to w1[0]+xc0",
                )
            for q in range(4):
                nc.sync.dma_start(
                    xT_sb[:, 1, q * QX : (q + 1) * QX], xc1[:, q * QX : (q + 1) * QX]
                )
            b1_sb = bc_sb[:, :FT]
            b2_sb = bc_sb[:, FT:]

            # PE warmup: dummy matmuls on scratch while the head DMAs stream.
            for i in range(N_WARMUP_MM):
                pw = psum_y.tile([P, 512], f32, tag="pm", bufs=4, name="pwarm")
                nc.tensor.matmul(
                    pw[:], scratch[:, :P], scratch[:], start=True, stop=True
                )

            h_sb = hpool.tile([P, FT, T], f16)
            # Strassen B-side combos of h: [P, combo(T1,T2,T3,T4), k, 512]
            t_sb = tpool.tile([P, 4, FH, 512], f16)

            # ---- phase 1: h^T = gelu(w1^T x^T + b1), one 128-row f-tile at a time
            def mm1_group(ph, w1_sb, c):
                for do in range(DO):
                    nc.tensor.matmul(
                        ph[:],
                        w1_sb[:, do, :],
                        xT_sb[:, c, do * 512 : (do + 1) * 512],
                        start=(do == 0),
                        stop=(do == DO - 1),
                    )

            def gelu_chunk(ph, ft, c):
                cs = slice(c * 512, (c + 1) * 512)
                return nc.scalar.activation(
                    h_sb[:, ft, cs], ph[:], gelu, bias=b1_sb[:, ft : ft + 1]
                )

            def t_combos(ft):
                # B11[i]=h[i,c0] B12[i]=h[i,c1] B21[i]=h[16+i,c0] B22[i]=h[16+i,c1]
                # T1=B12-B11 (at ft=i<16); T3=B22-B12, T2=B22-T1, T4=T2-B21
                # (at ft=16+i) — all on the otherwise-idle vector engine.
                if ft < FH:
                    i = ft
                    nc.vector.tensor_sub(
                        t_sb[:, 0, i, :], h_sb[:, i, 512:1024], h_sb[:, i, 0:512]
                    )
                else:
                    i = ft - FH
                    nc.vector.tensor_sub(
                        t_sb[:, 2, i, :], h_sb[:, ft, 512:1024], h_sb[:, i, 512:1024]
                    )
                    nc.vector.tensor_sub(
                        t_sb[:, 1, i, :], h_sb[:, ft, 512:1024], t_sb[:, 0, i, :]
                    )
                    nc.vector.tensor_sub(
                        t_sb[:, 3, i, :], t_sb[:, 1, i, :], h_sb[:, ft, 0:512]
                    )

            gelu_insts = {}
            HEAD = 3
            head_ph = {}
            for ft in range(HEAD):
                ph = psum_h.tile([P, 512], f32, tag="ph", bufs=4, name="ph")
                head_ph[ft] = ph
                mm1_group(ph, w1_tiles[ft], 0)
            for ft in range(HEAD):
                ph = head_ph[ft]
                gelu_insts[(ft, 0)] = gelu_chunk(ph, ft, 0)
                ph2 = psum_h.tile([P, 512], f32, tag="ph", bufs=4, name="ph")
                mm1_group(ph2, w1_tiles[ft], 1)
                gelu_insts[(ft, 1)] = gelu_chunk(ph2, ft, 1)
                t_combos(ft)

            for ft in range(HEAD, FT):
                if ft >= 5:
                    w1_tiles[ft] = wpool.tile(
                        [P, DO, P], f16, tag="w1", bufs=8, name="w1_sb"
                    )
                    nc.sync.dma_start(w1_tiles[ft][:], w1_d.ap()[ft])
                w1_sb = w1_tiles[ft]
                for c in range(NCHUNK):
                    ph = psum_h.tile([P, 512], f32, tag="ph", bufs=4, name="ph")
                    mm1_group(ph, w1_sb, c)
                    gelu_insts[(ft, c)] = gelu_chunk(ph, ft, c)
                t_combos(ft)

            # ---- phase 2: y^T = w2^T h^T + b2 via Strassen-Winograd.
            # Per unit j (output dm-tiles j and 4+j):
            #   M1=A11 B11  M2=A12 B21  M3=S4 B22  M4=A22 T4
            #   M5=S1 T1    M6=S2 T2    M7=S3 T3
            #   U2=M1+M6  U3=U2+M7  U4=U2+M5
            #   C11=M1+M2  C12=U4+M3  C21=U3-M4  C22=U3+M5   (+b2 folded in)
            # Mi order [1,2,6,7,5,3,4] releases each PSUM bank right after
            # its product completes.
            def moving(mi, k):
                if mi == 0:
                    return h_sb[:, k, 0:512]
                if mi == 1:
                    return h_sb[:, FH + k, 0:512]
                if mi == 2:
                    return h_sb[:, FH + k, 512:1024]
                combo = {4: 0, 5: 1, 6: 2, 3: 3}[mi]  # M5->T1 M6->T2 M7->T3 M4->T4
                return t_sb[:, combo, k, :]

            def mi_product(mi, j, n0=0, n1=512, pm=None):
                # accumulate Mi[j][:, n0:n1] over the 16 k-tiles; weights
                # stream in two [P, 8, P] slabs through the shared w1 pool
                if pm is None:
                    pm_t = psum_y.tile([P, 512], f32, tag="pm", bufs=4, name="pm")
                    pm = pm_t[:, n0:n1]
                for half in range(2):
                    slab = wpool.tile([P, FH // 2, P], f16, tag="w1", bufs=8,
                                      name="w2s_sb")
                    dma = nc.sync.dma_start(
                        slab[:],
                        w2_d.ap()[mi, j][:, half * 8 : (half + 1) * 8, :],
                    )
                    if mi == 0 and j == 0:
                        add_dep_helper(
                            dma.ins,
                            gelu_insts[(6, 1)].ins,
                            sync=True,
                            reason="delay w2 prefetch past the kernel head",
                        )
                    for kk in range(FH // 2):
                        k = half * 8 + kk
                        nc.tensor.matmul(
                            pm,
                            slab[:, kk, :],
                            moving(mi, k)[:, n0:n1],
                            start=(k == 0),
                            stop=(k == FH - 1),
                        )
                return pm

            def flush(src_sb, j_out, cs):
                nc.sync.dma_start(yT_d.ap()[j_out][:, cs], src_sb[:])

            for j in range(4):
                b2j = b2_sb[:, j : j + 1]
                b2j4 = b2_sb[:, 4 + j : 5 + j]
                pm1 = mi_product(0, j)
                s_m1 = spool.tile([P, 512], f32, tag="sm1", bufs=2, name="s_m1")
                nc.vector.tensor_copy(s_m1[:], pm1)
                pm2 = mi_product(1, j)
                y11 = ypool.tile([P, 512], f32, tag="y", bufs=4, name="y_sb")
                nc.vector.scalar_tensor_tensor(y11[:], s_m1[:], b2j, pm2, add, add)
                flush(y11, j, slice(0, 512))
                pm6 = mi_product(5, j)
                u2 = spool.tile([P, 512], f32, tag="u2", bufs=2, name="u2")
                nc.vector.tensor_add(u2[:], s_m1[:], pm6)
                pm7 = mi_product(6, j)
                u3 = spool.tile([P, 512], f32, tag="u3", bufs=2, name="u3")
                nc.vector.tensor_add(u3[:], u2[:], pm7)
                pm5 = mi_product(4, j)
                u4 = spool.tile([P, 512], f32, tag="u4", bufs=2, name="u4")
                nc.vector.tensor_add(u4[:], u2[:], pm5)
                y22 = ypool.tile([P, 512], f32, tag="y", bufs=4, name="y_sb")
                nc.vector.scalar_tensor_tensor(y22[:], u3[:], b2j4, pm5, add, add)
                flush(y22, 4 + j, slice(512, 1024))
                pm3 = mi_product(2, j)
                y12 = ypool.tile([P, 512], f32, tag="y", bufs=4, name="y_sb")
                nc.vector.scalar_tensor_tensor(y12[:], u4[:], b2j, pm3, add, add)
                flush(y12, j, slice(512, 1024))
                if j < 3:
                    pm4 = mi_product(3, j)
                    y21 = ypool.tile([P, 512], f32, tag="y", bufs=4, name="y_sb")
                    nc.vector.scalar_tensor_tensor(
                        y21[:], u3[:], b2j4, pm4, add, sub
                    )
                    flush(y21, 4 + j, slice(0, 512))
                else:
                    # last unit: M4 as two 256-wide groups so only a 256-col
                    # assembly+flush remains after the final matmul
                    pm4a_t = psum_y.tile([P, 512], f32, tag="pm", bufs=4, name="pm4a")
                    pm4a = pm4a_t[:, 0:256]
                    mi_product(3, j, 0, 256, pm4a)
                    y21a = ypool.tile([P, 256], f32, tag="yh", bufs=2, name="y21a")
                    nc.vector.scalar_tensor_tensor(
                        y21a[:], u3[:, 0:256], b2j4, pm4a, add, sub
                    )
                    flush(y21a, 4 + j, slice(0, 256))
                    pm4b_t = psum_y.tile([P, 512], f32, tag="pm", bufs=4, name="pm4b")
                    pm4b = pm4b_t[:, 0:256]
                    mi_product(3, j, 256, 512, pm4b)
                    y21b = ypool.tile([P, 256], f32, tag="yh", bufs=2, name="y21b")
                    nc.vector.scalar_tensor_tensor(
                        y21b[:], u3[:, 256:512], b2j4, pm4b, add, sub
                    )
                    nc.scalar.dma_start(yT_d.ap()[4 + j][:, 256:512], y21b[:])

    nc.compile()
    return nc


def _get_nc():
    global _cached
    if _cached is None:
        _cached = _build()
    return _cached


def make_in_maps(x, w1, b1, w2, b2):
    x = np.asarray(x, dtype=np.float32)
    w1 = np.asarray(w1, dtype=np.float32)
    b1 = np.asarray(b1, dtype=np.float32)
    w2 = np.asarray(w2, dtype=np.float32)
    b2 = np.asarray(b2, dtype=np.float32)

    tokens = x.reshape(E, T, D)
    in_maps = []
    for e in range(E):
        xT = np.ascontiguousarray(
            tokens[e].reshape(NCHUNK, 512, DO, P).transpose(0, 3, 2, 1)
        ).astype(np.float16)  # [c, p, do, t']
        w1r = np.ascontiguousarray(
            w1[e].reshape(DO, P, FT, P).transpose(2, 1, 0, 3)
        ).astype(np.float16)  # [ft, p, do, j]
        bc = np.ascontiguousarray(
            np.concatenate([b1[e].reshape(FT, P).T, b2[e].reshape(DMO, P).T], axis=1)
        )  # [p, ft..dmo]
        # Strassen A-side combos of w2 [F, D]:
        w2e = w2[e]
        A11 = w2e[: F // 2, : D // 2]
        A12 = w2e[F // 2 :, : D // 2]
        A21 = w2e[: F // 2, D // 2 :]
        A22 = w2e[F // 2 :, D // 2 :]
        S1 = A21 + A22
        S2 = S1 - A11
        S3 = A11 - A21
        S4 = A12 - S2
        W = np.stack([A11, A12, S4, A22, S1, S2, S3])  # [7, F/2, D/2]
        w2s = np.ascontiguousarray(
            W.reshape(7, FH, P, 4, P).transpose(0, 3, 2, 1, 4)
        ).astype(np.float16)  # [mi, j, p, k, dm]
        in_maps.append({"xT": xT, "w1r": w1r, "bc": bc, "w2s": w2s})
    return in_maps


def gather_out(results):
    out = np.empty((E, T, D), dtype=np.float32)
    for e in range(E):
        yT = results[e]["yT"]  # [dmo, p, t]
        out[e] = yT.transpose(2, 0, 1).reshape(T, D)
    return out.reshape(B, S, D)


def kernel(x, w1, b1, w2, b2):
    from concourse.bass_utils import run_bass_kernel_spmd

    nc = _get_nc()
    in_maps = make_in_maps(x, w1, b1, w2, b2)
    res = run_bass_kernel_spmd(nc, in_maps, core_ids=list(range(E)))
    return gather_out(res.results)


# revision 13
# speedup vs baseline: 1.0215x; 1.0003x over previous
"""Trainium2 Bass kernel for an 8-expert MoE FFN layer (nn_MoELayer).

Reference computation (per expert e over its contiguous 1024-token chunk):
    h = gelu(x_e @ w1[e] + b1[e]);  y_e = h @ w2[e] + b2[e]

Sharding: expert parallelism — core e holds expert e's weights and its token
chunk (the gate yields equal contiguous chunks, so no all-to-all is needed).
Each core runs the same SPMD program on its own data.

Per-core kernel (T=1024 tokens, D=1024, F=4096), all matmuls in fp16 with
fp32 PSUM accumulation (~216 ns per 512-wide matmul, the PE floor):

  phase 1: for each 128-wide f-tile: h^T[ft] = gelu(w1[:,ft]^T @ x^T + b1[ft])
           (f on partitions -> b1 is a per-partition ACT bias; h^T resident in
           SBUF).  While phase 1 runs, the vector engine (otherwise idle)
           builds the Strassen B-side combos of h for phase 2.

  phase 2: y^T = w2^T h^T + b2 via one level of Strassen-Winograd over the
           block split [D/2, F/2, T/2]: 7 products Mi of shape [512, 512]
           instead of 8 — 448 matmuls instead of 512.  The A-side (w2) combos
           are prepared on the host; the B-side combos T1..T4 of h are built
           by the vector engine during phase 1; the C-side assembly runs on
           the vector engine (tensor_tensor + fused scalar_tensor_tensor with
           the b2 bias), overlapped with the PE.

Head: the first ~1.3 MiB (w1[0] + x chunk 0) gates the matmul stream, and the
DMA path is latency-bound (~200-250 GB/s) for the first few microseconds.
The critical pieces stream on the sync/SP HWDGE ring in fine slices (per-DMA
completion semaphores let each matmul start as its slice lands); secondary
pieces ride the gpsimd SWDGE ring and the scalar/ACT HWDGE ring (which has a
~2.8 us cold-start lag).  Dummy matmuls on scratch bridge the PE from the
preamble barrier to the first real matmul and open the HAM clock-warmup
window early.

Tail: the last product of the last Strassen unit runs as two 256-wide PSUM
groups so only a 256-column assembly+flush remains after the final matmul.
"""

import os

import numpy as np

# The kernel executes through the axon PJRT backend; a CPU pin (e.g. set for
# a jax reference run) would break NEFF dispatch in this process.
if os.environ.get("JAX_PLATFORMS") == "cpu":
    del os.environ["JAX_PLATFORMS"]

E = 8          # experts == cores
B, S = 2, 4096
D = 1024       # d_model
F = 4096       # d_ff
T = (B * S) // E  # tokens per expert chunk = 1024
P = 128
DO = D // P    # 8  k-tiles of d_model
FT = F // P    # 32 f-tiles of d_ff
DMO = D // P   # 8  output dm-tiles
FH = FT // 2   # 16 f-tiles per Strassen half of d_ff
NCHUNK = T // 512  # 2 moving-operand chunks (PSUM bank caps matmul N at 512)
N_WARMUP_MM = 16

_cached = None


def _build():
    import concourse.mybir as mybir
    import concourse.tile as tile
    from concourse import bacc
    from concourse.tile_rust import add_dep_helper

    f32 = mybir.dt.float32
    f16 = mybir.dt.float16
    add = mybir.AluOpType.add
    sub = mybir.AluOpType.subtract

    nc = bacc.Bacc("TRN2", target_bir_lowering=False, debug=False, num_devices=E)

    xT_d = nc.dram_tensor("xT", [NCHUNK, P, DO, 512], f16, kind="ExternalInput")
    w1_d = nc.dram_tensor("w1r", [FT, P, DO, P], f16, kind="ExternalInput")
    bc_d = nc.dram_tensor("bc", [P, FT + DMO], f32, kind="ExternalInput")
    # Strassen A-side operands for phase 2: [mi, j, p(f-within-half), k, dm]
    w2_d = nc.dram_tensor("w2s", [7, 4, P, FH, P], f16, kind="ExternalInput")
    yT_d = nc.dram_tensor("yT", [DMO, P, T], f32, kind="ExternalOutput")

    gelu = mybir.ActivationFunctionType.Gelu_apprx_tanh

    with tile.TileContext(nc) as tc:
        with (
            tc.tile_pool(name="xpool", bufs=1) as xpool,
            tc.tile_pool(name="hpool", bufs=1) as hpool,
            tc.tile_pool(name="tpool", bufs=1) as tpool,
            tc.tile_pool(name="wpool", bufs=2) as wpool,
            tc.tile_pool(name="cpool", bufs=1) as cpool,
            tc.tile_pool(name="spool", bufs=2) as spool,
            tc.tile_pool(name="ypool", bufs=2) as ypool,
            tc.tile_pool(name="psum_h", bufs=2, space="PSUM") as psum_h,
            tc.tile_pool(name="psum_y", bufs=2, space="PSUM") as psum_y,
        ):
            # scratch for PE warmup: direct f16 memset on the vector engine
            scratch = cpool.tile([P, 512], f16)
            nc.vector.memset(scratch[:], 0.0)

            # ---- head input DMAs.  Early DMA throughput is latency-bound at
            # ~200-250 GB/s TOTAL across all rings (the SDMA engines are
            # shared, packets round-robin across queues), so priority comes
            # from keeping the early window exclusive: the critical sequence
            # w1[0] -> x chunk-0 slices streams alone on the sync/SP ring;
            # every secondary piece (gpsimd SWDGE / scalar ACT rings) is
            # gated behind an x-slice completion so it cannot steal early
            # bandwidth.  Per-slice completion sems let each matmul start as
            # its slice lands.
            w1_tiles = {}
            for ft in range(5):
                w1_tiles[ft] = wpool.tile(
                    [P, DO, P], f16, tag="w1", bufs=8, name="w1_sb"
                )
            xT_sb = xpool.tile([P, NCHUNK, DO * 512], f16)
            xc0 = xT_d.ap()[0].rearrange("p do t -> p (do t)")
            xc1 = xT_d.ap()[1].rearrange("p do t -> p (do t)")
            QX = DO * 512 // 4  # do-pair slice: 1024 fp16 per partition

            bc_sb = cpool.tile([P, FT + DMO], f32)
            nc.sync.dma_start(w1_tiles[0][:], w1_d.ap()[0])
            xs = []
            for q in range(4):
                xs.append(
                    nc.sync.dma_start(
                        xT_sb[:, 0, q * QX : (q + 1) * QX],
                        xc0[:, q * QX : (q + 1) * QX],
                    )
                )
            nc.sync.dma_start(w1_tiles[1][:], w1_d.ap()[1])
            nc.sync.dma_start(w1_tiles[2][:], w1_d.ap()[2])
            # biases are tiny — let them ride the idle SWDGE ring immediately
            nc.gpsimd.dma_start(bc_sb[:], bc_d.ap())
            for dma, dep in (
                (nc.scalar.dma_start(w1_tiles[3][:], w1_d.ap()[3]), xs[1]),
                (nc.scalar.dma_start(w1_tiles[4][:], w1_d.ap()[4]), xs[2]),
            ):
                add_dep_helper(
                    dma.ins, dep.ins, sync=True,
                    reason="keep the early DMA window exclusive to w1[0]+xc0",
                )
            for q in range(4):
                nc.sync.dma_start(
                    xT_sb[:, 1, q * QX : (q + 1) * QX], xc1[:, q * QX : (q + 1) * QX]
                )
            b1_sb = bc_sb[:, :FT]
            b2_sb = bc_sb[:, FT:]

            # PE warmup: dummy matmuls on scratch while the head DMAs stream.
            for i in range(N_WARMUP_MM):
                pw = psum_y.tile([P, 512], f32, tag="pm", bufs=4, name="pwarm")
                nc.tensor.matmul(
                    pw[:], scratch[:, :P], scratch[:], start=True, stop=True
                )

            h_sb = hpool.tile([P, FT, T], f16)
            # Strassen B-side combos of h: [P, combo(T1,T2,T3,T4), k, 512]
            t_sb = tpool.tile([P, 4, FH, 512], f16)

            # ---- phase 1: h^T = gelu(w1^T x^T + b1), one 128-row f-tile at a time
            def mm1_group(ph, w1_sb, c):
                for do in range(DO):
                    nc.tensor.matmul(
                        ph[:],
                        w1_sb[:, do, :],
                        xT_sb[:, c, do * 512 : (do + 1) * 512],
                        start=(do == 0),
                        stop=(do == DO - 1),
                    )

            def gelu_chunk(ph, ft, c):
                cs = slice(c * 512, (c + 1) * 512)
                return nc.scalar.activation(
                    h_sb[:, ft, cs], ph[:], gelu, bias=b1_sb[:, ft : ft + 1]
                )

            def t_combos(ft):
                # B11[i]=h[i,c0] B12[i]=h[i,c1] B21[i]=h[16+i,c0] B22[i]=h[16+i,c1]
                # T1=B12-B11 (at ft=i<16); T3=B22-B12, T2=B22-T1, T4=T2-B21
                # (at ft=16+i) — all on the otherwise-idle vector engine.
                if ft < FH:
                    i = ft
                    nc.vector.tensor_sub(
                        t_sb[:, 0, i, :], h_sb[:, i, 512:1024], h_sb[:, i, 0:512]
                    )
                else:
                    i = ft - FH
                    nc.vector.tensor_sub(
                        t_sb[:, 2, i, :], h_sb[:, ft, 512:1024], h_sb[:, i, 512:1024]
                    )
                    nc.vector.tensor_sub(
                        t_sb[:, 1, i, :], h_sb[:, ft, 512:1024], t_sb[:, 0, i, :]
                    )
                    nc.vector.tensor_sub(
                        t_sb[:, 3, i, :], t_sb[:, 1, i, :], h_sb[:, ft, 0:512]
                    )

            gelu_insts = {}
            HEAD = 5
            head_ph = {}
            for ft in range(HEAD):
                ph = psum_h.tile([P, 512], f32, tag="ph", bufs=4, name="ph")
                head_ph[ft] = ph
                mm1_group(ph, w1_tiles[ft], 0)
            for ft in range(HEAD):
                ph = head_ph[ft]
                gelu_insts[(ft, 0)] = gelu_chunk(ph, ft, 0)
                ph2 = psum_h.tile([P, 512], f32, tag="ph", bufs=4, name="ph")
                mm1_group(ph2, w1_tiles[ft], 1)
                gelu_insts[(ft, 1)] = gelu_chunk(ph2, ft, 1)
                t_combos(ft)

            for ft in range(HEAD, FT):
                if ft >= 5:
                    w1_tiles[ft] = wpool.tile(
                        [P, DO, P], f16, tag="w1", bufs=8, name="w1_sb"
                    )
                    nc.sync.dma_start(w1_tiles[ft][:], w1_d.ap()[ft])
                w1_sb = w1_tiles[ft]
                for c in range(NCHUNK):
                    ph = psum_h.tile([P, 512], f32, tag="ph", bufs=4, name="ph")
                    mm1_group(ph, w1_sb, c)
                    gelu_insts[(ft, c)] = gelu_chunk(ph, ft, c)
                t_combos(ft)

            # ---- phase 2: y^T = w2^T h^T + b2 via Strassen-Winograd.
            # Per unit j (output dm-tiles j and 4+j):
            #   M1=A11 B11  M2=A12 B21  M3=S4 B22  M4=A22 T4
            #   M5=S1 T1    M6=S2 T2    M7=S3 T3
            #   U2=M1+M6  U3=U2+M7  U4=U2+M5
            #   C11=M1+M2  C12=U4+M3  C21=U3-M4  C22=U3+M5   (+b2 folded in)
            # Mi order [1,2,6,7,5,3,4] releases each PSUM bank right after
            # its product completes.
            def moving(mi, k):
                if mi == 0:
                    return h_sb[:, k, 0:512]
                if mi == 1:
                    return h_sb[:, FH + k, 0:512]
                if mi == 2:
                    return h_sb[:, FH + k, 512:1024]
                combo = {4: 0, 5: 1, 6: 2, 3: 3}[mi]  # M5->T1 M6->T2 M7->T3 M4->T4
                return t_sb[:, combo, k, :]

            def mi_product(mi, j, n0=0, n1=512, pm=None):
                # accumulate Mi[j][:, n0:n1] over the 16 k-tiles; weights
                # stream in two [P, 8, P] slabs through the shared w1 pool
                if pm is None:
                    pm_t = psum_y.tile([P, 512], f32, tag="pm", bufs=4, name="pm")
                    pm = pm_t[:, n0:n1]
                for half in range(2):
                    slab = wpool.tile([P, FH // 2, P], f16, tag="w1", bufs=8,
                                      name="w2s_sb")
                    dma = nc.sync.dma_start(
                        slab[:],
                        w2_d.ap()[mi, j][:, half * 8 : (half + 1) * 8, :],
                    )
                    if mi == 0 and j == 0:
                        add_dep_helper(
                            dma.ins,
                            gelu_insts[(6, 1)].ins,
                            sync=True,
                            reason="delay w2 prefetch past the kernel head",
                        )
                    for kk in range(FH // 2):
                        k = half * 8 + kk
                        nc.tensor.matmul(
                            pm,
                            slab[:, kk, :],
                            moving(mi, k)[:, n0:n1],
                            start=(k == 0),
                            stop=(k == FH - 1),
                        )
                return pm

            def flush(src_sb, j_out, cs):
                nc.sync.dma_start(yT_d.ap()[j_out][:, cs], src_sb[:])

            for j in range(4):
                b2j = b2_sb[:, j : j + 1]
                b2j4 = b2_sb[:, 4 + j : 5 + j]
                pm1 = mi_product(0, j)
                s_m1 = spool.tile([P, 512], f32, tag="sm1", bufs=2, name="s_m1")
                nc.vector.tensor_copy(s_m1[:], pm1)
                pm2 = mi_product(1, j)
                y11 = ypool.tile([P, 512], f32, tag="y", bufs=4, name="y_sb")
                nc.vector.scalar_tensor_tensor(y11[:], s_m1[:], b2j, pm2, add, add)
                flush(y11, j, slice(0, 512))
                pm6 = mi_product(5, j)
                u2 = spool.tile([P, 512], f32, tag="u2", bufs=2, name="u2")
                nc.vector.tensor_add(u2[:], s_m1[:], pm6)
                pm7 = mi_product(6, j)
                u3 = spool.tile([P, 512], f32, tag="u3", bufs=2, name="u3")
                nc.vector.tensor_add(u3[:], u2[:], pm7)
                pm5 = mi_product(4, j)
                u4 = spool.tile([P, 512], f32, tag="u4", bufs=2, name="u4")
                nc.vector.tensor_add(u4[:], u2[:], pm5)
                y22 = ypool.tile([P, 512], f32, tag="y", bufs=4, name="y_sb")
                nc.vector.scalar_tensor_tensor(y22[:], u3[:], b2j4, pm5, add, add)
                flush(y22, 4 + j, slice(512, 1024))
                pm3 = mi_product(2, j)
                y12 = ypool.tile([P, 512], f32, tag="y", bufs=4, name="y_sb")
                nc.vector.scalar_tensor_tensor(y12[:], u4[:], b2j, pm3, add, add)
                flush(y12, j, slice(512, 1024))
                if j < 3:
                    pm4 = mi_product(3, j)
                    y21 = ypool.tile([P, 512], f32, tag="y", bufs=4, name="y_sb")
                    nc.vector.scalar_tensor_tensor(
                        y21[:], u3[:], b2j4, pm4, add, sub
                    )
                    flush(y21, 4 + j, slice(0, 512))
                else:
                    # last unit: M4 as two 256-wide groups so only a 256-col
                    # assembly+flush remains after the final matmul
                    pm4a_t = psum_y.tile([P, 512], f32, tag="pm", bufs=4, name="pm4a")
                    pm4a = pm4a_t[:, 0:256]
                    mi_product(3, j, 0, 256, pm4a)
                    y21a = ypool.tile([P, 256], f32, tag="yh", bufs=2, name="y21a")
                    nc.vector.scalar_tensor_tensor(
                        y21a[:], u3[:, 0:256], b2j4, pm4a, add, sub
                    )
                    flush(y21a, 4 + j, slice(0, 256))
                    pm4b_t = psum_y.tile([P, 512], f32, tag="pm", bufs=4, name="pm4b")
                    pm4b = pm4b_t[:, 0:256]
                    mi_product(3, j, 256, 512, pm4b)
                    y21b = ypool.tile([P, 256], f32, tag="yh", bufs=2, name="y21b")
                    nc.vector.scalar_tensor_tensor(
                        y21b[:], u3[:, 256:512], b2j4, pm4b, add, sub
                    )
                    nc.scalar.dma_start(yT_d.ap()[4 + j][:, 256:512], y21b[:])

    nc.compile()
    return nc


def _get_nc():
    global _cached
    if _cached is None:
        _cached = _build()
    return _cached


def make_in_maps(x, w1, b1, w2, b2):
    x = np.asarray(x, dtype=np.float32)
    w1 = np.asarray(w1, dtype=np.float32)
    b1 = np.asarray(b1, dtype=np.float32)
    w2 = np.asarray(w2, dtype=np.float32)
    b2 = np.asarray(b2, dtype=np.float32)

    tokens = x.reshape(E, T, D)
    in_maps = []
    for e in range(E):
        xT = np.ascontiguousarray(
            tokens[e].reshape(NCHUNK, 512, DO, P).transpose(0, 3, 2, 1)
        ).astype(np.float16)  # [c, p, do, t']
        w1r = np.ascontiguousarray(
            w1[e].reshape(DO, P, FT, P).transpose(2, 1, 0, 3)
        ).astype(np.float16)  # [ft, p, do, j]
        bc = np.ascontiguousarray(
            np.concatenate([b1[e].reshape(FT, P).T, b2[e].reshape(DMO, P).T], axis=1)
        )  # [p, ft..dmo]
        # Strassen A-side combos of w2 [F, D]:
        w2e = w2[e]
        A11 = w2e[: F // 2, : D // 2]
        A12 = w2e[F // 2 :, : D // 2]
        A21 = w2e[: F // 2, D // 2 :]
        A22 = w2e[F // 2 :, D // 2 :]
        S1 = A21 + A22
        S2 = S1 - A11
        S3 = A11 - A21
        S4 = A12 - S2
        W = np.stack([A11, A12, S4, A22, S1, S2, S3])  # [7, F/2, D/2]
        w2s = np.ascontiguousarray(
            W.reshape(7, FH, P, 4, P).transpose(0, 3, 2, 1, 4)
        ).astype(np.float16)  # [mi, j, p, k, dm]
        in_maps.append({"xT": xT, "w1r": w1r, "bc": bc, "w2s": w2s})
    return in_maps


def gather_out(results):
    out = np.empty((E, T, D), dtype=np.float32)
    for e in range(E):
        yT = results[e]["yT"]  # [dmo, p, t]
        out[e] = yT.transpose(2, 0, 1).reshape(T, D)
    return out.reshape(B, S, D)


def kernel(x, w1, b1, w2, b2):
    from concourse.bass_utils import run_bass_kernel_spmd

    nc = _get_nc()
    in_maps = make_in_maps(x, w1, b1, w2, b2)
    res = run_bass_kernel_spmd(nc, in_maps, core_ids=list(range(E)))
    return gather_out(res.results)
